# revision 12
# baseline (speedup 1.0000x reference)
"""Born-Wolf PSF kernel for Trainium2, 8 NeuronCores, data-parallel over batch.

Self-contained: hardcodes all geometry from the problem spec.
  input : params (16, 64, 2) float32
  output: psf    (16, 64, 25, 25, 25) float32

Per (b,c) pair: psf = |trapz_rho J0(k n r rho) exp(-i 0.5 k rho^2 z n^2) rho|^2,
bilinearly interpolated from 35 anchor radii onto a 25x25 grid, reflect-padded
in z, and normalized.

v2 strategy: all per-(rho, pair*anchor) field quantities are sums of outer
products (rank-k separable), so they are computed by TensorE matmuls directly
into PSUM instead of DRAM-row broadcast DMAs (which serialized ~375us on one
DMA engine in v1). f32r matmuls truncate inputs to ~12 mantissa bits, so the
large phase factors (up to ~145 turns) are split hi/lo: hi holds 10 explicit
bits (exact in f32r), lo carries the remainder; the rank-5 expansion
(ones, hi*hi, hi*lo, lo*hi, lo*lo) restores full fp32 accuracy at 1 cycle/col.
Trapezoid weights and 1/2pi factors are folded into the matmul lhs constants;
normalization is folded into the per-partition scale of the PSUM->SBUF copy
after the G-expansion matmul.
"""
import os
import numpy as np

# ---------------- problem geometry (hardcoded) ----------------
B, CH = 16, 64
NCORES = 8
NP = (B // NCORES) * CH          # 128 pairs per core
NA, NJ, NZH, NZ = 35, 101, 13, 25
F = NP * NA                      # 4480
FCH = NP * NZH                   # 1664
NYX = 625
GW = 640                         # zero-padded G columns (2 x 320 matmuls)
PI = float(np.pi)
C0 = -0.1562499995e-1
C1 = -0.1098628627e-2
C2 = 0.1430488765e-3
S_AMP = float(np.sqrt(0.636619772))
SC1 = 20.0                       # power-row scale split (clamp = SC1^2 = 400)
QS = [1.0, -0.25, 0.015624999996, -0.00043402777473, 6.7816828549e-06,
      -6.781657507e-08, 4.7091319698e-10, -2.3995591574e-12,
      9.2118377553e-15, -2.3695100804e-17]
MAGIC = 12582912.0               # 1.5 * 2**23: (t+M)-M == round-to-nearest(t)
TMASK = 4.0 / (2.0 * PI) + 0.125  # mask threshold in turn units
VC = 8193.0                      # Veltkamp split const (2^13+1): hi keeps 10 bits

_CACHE = {}


def _split10(x):
    """Split f32 values into (hi, lo); hi has <=10 explicit mantissa bits."""
    xf = np.ascontiguousarray(np.asarray(x, np.float32))
    hi = (xf.view(np.uint32) & np.uint32(0xFFFFE000)).view(np.float32).copy()
    lo = (xf - hi).astype(np.float32)
    return hi, lo


def _host_consts():
    if "consts" in _CACHE:
        return _CACHE["consts"]
    f32 = np.float32
    R = (np.linspace(0, 34, NA) / 2.0).astype(np.float64)          # anchor radii
    RHO = np.linspace(0.0, 1.0, NJ).astype(np.float64)
    yp = xp = 12.0
    Y, X = np.meshgrid(np.arange(25.0), np.arange(25.0), indexing="ij")
    rPix = np.sqrt((X - xp) ** 2 + (Y - yp) ** 2)
    IDX1 = np.floor(rPix * 2).astype(np.int32)
    IDX2 = IDX1 + 1
    DISR1 = ((rPix - R[IDX1]) * 2).astype(np.float32).astype(np.float64)
    DISR2 = 1.0 - DISR1

    G = np.zeros((NA, GW), np.float64)
    for yy in range(25):
        for xx in range(25):
            yx = yy * 25 + xx
            G[IDX2[yy, xx], yx] += DISR1[yy, xx]
            G[IDX1[yy, xx], yx] += DISR2[yy, xx]
    gcol = G[:, :NYX].sum(1)
    w13 = np.concatenate([[1.0], np.full(NZH - 1, 2.0)])
    gw13 = np.zeros((NA, 2 * NZH))
    gw13[:, 0::2] = gcol[:, None] * w13[None, :]

    wt = np.full(NJ, 0.01, np.float64)
    wt[0] *= 0.5
    wt[-1] *= 0.5
    rw = RHO * wt                                    # trapezoid weight * rho

    with np.errstate(divide="ignore"):
        rinv = 1.0 / RHO
    rinv[0] = 0.0

    # phase lhs: t0 = 0.125 + RHO*knr_t + ps1*rinv/(2pi)^2 + ps3*rinv^3/(2pi)^4
    rho_hi, rho_lo = _split10(RHO)
    L_ph1 = np.stack([np.full(NJ, 0.125), rho_hi, rho_hi, rho_lo, rho_lo])
    L_ph2 = np.stack([rinv / (2 * np.pi) ** 2, rinv ** 3 / (2 * np.pi) ** 4])

    # poly lhs (J0 small branch, trapezoid weight folded in); row 0 <-> ones
    qrho = np.stack([QS[m] * (SC1 * SC1 * RHO ** 2) ** m for m in range(10)])
    L_poly = qrho * rw[None, :]

    # amplitude lhs (J0 large branch, trapezoid weight folded in)
    L_amp = np.stack([np.sqrt(rinv) * rw, -rinv ** 2.5 * rw])

    # C-matrix lhs: turns = RHO^2 * wcz_t  (+0.25 for cos; negated for sin)
    r2hi, r2lo = _split10(RHO ** 2)
    L_cos = np.stack([np.full(NJ, 0.25), r2hi, r2hi, r2lo, r2lo])
    L_sin = np.stack([np.zeros(NJ), -r2hi, -r2hi, -r2lo, -r2lo])

    Rinv = 1.0 / np.maximum(R, 1e-9)
    Rinv[0] = 0.0
    rtab = np.tile(R[None, :], (NP, 1))
    ri8c0 = np.tile((8.0 * C0 * Rinv)[None, :], (NP, 1))
    ri3c2 = np.tile((512.0 * C2 * Rinv ** 3)[None, :], (NP, 1))
    ztab = np.tile(np.arange(NZH, dtype=np.float64)[None, :], (NP, 1))

    consts = {
        "Lph1": L_ph1.astype(f32),
        "Lph2": L_ph2.astype(f32),
        "Lpoly": L_poly.astype(f32),
        "Lamp": L_amp.astype(f32),
        "Lcos": L_cos.astype(f32),
        "Lsin": L_sin.astype(f32),
        "rtab": rtab.astype(f32),
        "ri8c0": ri8c0.astype(f32),
        "ri3c2": ri3c2.astype(f32),
        "ztab": ztab.astype(f32),
        "gpad": G.astype(f32),
        "gw13": gw13.astype(f32),
    }
    for k, v in consts.items():
        assert np.isfinite(v).all(), k
    _CACHE["consts"] = consts
    return consts


def _ensure_paths():
    import sys
    for p in ("/opt/trn_rl_repo", "/root/.axon_site/_ro/trn_rl_repo"):
        if os.path.isdir(p) and p not in sys.path:
            sys.path.append(p)


def _build_nc():
    if "nc" in _CACHE:
        return _CACHE["nc"]
    _ensure_paths()
    from contextlib import ExitStack
    import concourse.bass as bass
    import concourse.bacc as bacc
    import concourse.tile as tile
    from concourse import mybir

    f32 = mybir.dt.float32
    f32r = mybir.dt.float32r
    bf16 = mybir.dt.bfloat16
    u8 = mybir.dt.uint8
    AF = mybir.ActivationFunctionType
    OP = mybir.AluOpType

    nc = bacc.Bacc()
    BIAS_A1 = float(np.log(S_AMP) - 0.5 * np.log(2 * np.pi))
    BIAS_A2 = float(np.log(64.0 * abs(C1) * S_AMP) - 2.5 * np.log(2 * np.pi))
    for val in (BIAS_A1, BIAS_A2):
        t = nc.alloc_sbuf_tensor(f"const-f32-{val}", [128, 1], f32)
        nc.gpsimd.memset(t.ap(), val)
        nc.const_aps.aps[(f32, val)] = t.ap()
    nc.all_engine_barrier()

    d_par = nc.declare_dram_parameter("params", [NP, 2], f32, isOutput=False)
    d_lph1 = nc.declare_dram_parameter("Lph1", [5, NJ], f32, isOutput=False)
    d_lph2 = nc.declare_dram_parameter("Lph2", [2, NJ], f32, isOutput=False)
    d_lpoly = nc.declare_dram_parameter("Lpoly", [10, NJ], f32, isOutput=False)
    d_lamp = nc.declare_dram_parameter("Lamp", [2, NJ], f32, isOutput=False)
    d_lcos = nc.declare_dram_parameter("Lcos", [5, NJ], f32, isOutput=False)
    d_lsin = nc.declare_dram_parameter("Lsin", [5, NJ], f32, isOutput=False)
    d_rtab = nc.declare_dram_parameter("rtab", [NP, NA], f32, isOutput=False)
    d_ri1 = nc.declare_dram_parameter("ri8c0", [NP, NA], f32, isOutput=False)
    d_ri3 = nc.declare_dram_parameter("ri3c2", [NP, NA], f32, isOutput=False)
    d_z = nc.declare_dram_parameter("ztab", [NP, NZH], f32, isOutput=False)
    d_g = nc.declare_dram_parameter("gpad", [NA, GW], f32, isOutput=False)
    d_gw = nc.declare_dram_parameter("gw13", [NA, 2 * NZH], f32,
                                     isOutput=False)
    d_out = nc.declare_dram_parameter("out", [NP, NZ, NYX], f32, isOutput=True)

    with tile.TileContext(nc) as tc, ExitStack() as ctx:
        p1 = ctx.enter_context(tc.tile_pool(name="p1", bufs=1))
        p2 = ctx.enter_context(tc.tile_pool(name="p2", bufs=3))

        # ---- const loads ----
        t_par = p1.tile([NP, 2], f32, tag="par")
        t_lph1f = p1.tile([5, NJ], f32, tag="lph1f")
        t_lph2f = p1.tile([2, NJ], f32, tag="lph2f")
        t_lpolyf = p1.tile([10, NJ], f32, tag="lpolyf")
        t_lampf = p1.tile([2, NJ], f32, tag="lampf")
        t_lcosf = p1.tile([5, NJ], f32, tag="lcosf")
        t_lsinf = p1.tile([5, NJ], f32, tag="lsinf")
        t_rtab = p1.tile([NP, NA], f32, tag="rtab")
        t_ri1 = p1.tile([NP, NA], f32, tag="ri1")
        t_ri3 = p1.tile([NP, NA], f32, tag="ri3")
        t_z = p1.tile([NP, NZH], f32, tag="ztab")
        t_gf = p1.tile([NA, GW], f32, tag="gpadf")
        t_gwf = p1.tile([NA, 2 * NZH], f32, tag="gw13f")
        for t, d in ((t_par, d_par), (t_lph1f, d_lph1), (t_lph2f, d_lph2),
                     (t_lpolyf, d_lpoly), (t_lampf, d_lamp), (t_lcosf, d_lcos),
                     (t_lsinf, d_lsin), (t_rtab, d_rtab), (t_ri1, d_ri1),
                     (t_ri3, d_ri3), (t_z, d_z), (t_gf, d_g), (t_gwf, d_gw)):
            nc.sync.dma_start(out=t[:], in_=d[:])
        t_lph1 = p1.tile([5, NJ], f32r, tag="lph1")
        t_lph2 = p1.tile([2, NJ], f32r, tag="lph2")
        t_lpoly = p1.tile([10, NJ], f32r, tag="lpoly")
        t_lamp = p1.tile([2, NJ], f32r, tag="lamp")
        t_lcos = p1.tile([5, NJ], f32r, tag="lcos")
        t_lsin = p1.tile([5, NJ], f32r, tag="lsin")
        t_g = p1.tile([NA, GW], f32r, tag="gpad")
        t_gw = p1.tile([NA, 2 * NZH], f32r, tag="gw13")
        for dst, srcf in ((t_lph1, t_lph1f), (t_lph2, t_lph2f),
                          (t_lpoly, t_lpolyf), (t_lamp, t_lampf),
                          (t_lcos, t_lcosf), (t_lsin, t_lsinf),
                          (t_g, t_gf), (t_gw, t_gwf)):
            nc.vector.tensor_copy(dst[:], srcf[:])

        # ---- pair-scalar stage ([NP,1] / [NP,NA]) ----
        t_abs = p1.tile([NP, 2], f32, tag="pabs")
        nc.vector.scalar_tensor_tensor(t_abs[:], t_par[:], -1.0, t_par[:],
                                       OP.mult, OP.max)
        lam = t_abs[:, 0:1]
        enn = t_abs[:, 1:2]
        t_rl = p1.tile([NP, 1], f32, tag="rl")
        nc.vector.reciprocal(t_rl[:], lam)
        t_knt = p1.tile([NP, 1], f32, tag="knt")       # n/lam (turns per R*rho)
        nc.vector.tensor_tensor(t_knt[:], enn, t_rl[:], OP.mult)
        t_rkn = p1.tile([NP, 1], f32, tag="rkn")       # lam/n
        nc.vector.reciprocal(t_rkn[:], t_knt[:])
        t_rkn3 = p1.tile([NP, 1], f32, tag="rkn3")
        nc.vector.tensor_tensor(t_rkn3[:], t_rkn[:], t_rkn[:], OP.mult)
        nc.vector.tensor_tensor(t_rkn3[:], t_rkn3[:], t_rkn[:], OP.mult)
        t_wct = p1.tile([NP, 1], f32, tag="wct")       # 0.5*n^2/lam
        nc.vector.scalar_tensor_tensor(t_wct[:], enn, 0.5, t_knt[:],
                                       OP.mult, OP.mult)

        t_knr = p1.tile([NP, NA], f32, tag="knr")      # knr in turns, <=145
        nc.vector.tensor_scalar(t_knr[:], t_rtab[:], t_knt[:], None, OP.mult)
        # Veltkamp split: hi keeps ~10 bits (exact under f32r truncation)
        t_kv = p1.tile([NP, NA], f32, tag="kv")
        nc.vector.tensor_scalar(t_kv[:], t_knr[:], VC, None, OP.mult)
        t_kz = p1.tile([NP, NA], f32, tag="kz")
        nc.vector.tensor_tensor(t_kz[:], t_kv[:], t_knr[:], OP.subtract)
        t_khi = p1.tile([NP, NA], f32, tag="khi")
        nc.vector.tensor_tensor(t_khi[:], t_kv[:], t_kz[:], OP.subtract)
        t_klo = p1.tile([NP, NA], f32, tag="klo")
        nc.vector.tensor_tensor(t_klo[:], t_knr[:], t_khi[:], OP.subtract)

        t_ps1 = p1.tile([NP, NA], f32, tag="ps1")
        nc.vector.tensor_scalar(t_ps1[:], t_ri1[:], t_rkn[:], None, OP.mult)
        t_ps3 = p1.tile([NP, NA], f32, tag="ps3")
        nc.vector.tensor_scalar(t_ps3[:], t_ri3[:], t_rkn3[:], None, OP.mult)

        t_knm = p1.tile([NP, NA], f32, tag="knm")
        nc.vector.tensor_scalar_max(t_knm[:], t_knr[:], 1e-4)
        t_lk = p1.tile([NP, NA], f32, tag="lk")
        nc.scalar.activation(t_lk[:], t_knm[:], AF.Ln)
        t_a1 = p1.tile([NP, NA], f32, tag="a1")
        nc.scalar.activation(t_a1[:], t_lk[:], AF.Exp, bias=BIAS_A1, scale=-0.5)
        t_a2 = p1.tile([NP, NA], f32, tag="a2")
        nc.scalar.activation(t_a2[:], t_lk[:], AF.Exp, bias=BIAS_A2, scale=-2.5)

        # power rows: v = min(knr_rad/SC1, SC1)^2 ; U[:, m*NA:(m+1)*NA] = v^(m+1)
        t_v0 = p1.tile([NP, NA], f32, tag="v0")
        nc.vector.tensor_scalar(t_v0[:], t_knr[:], 2.0 * PI / SC1, SC1,
                                OP.mult, OP.min)
        t_U = p1.tile([NP, 9 * NA], f32, tag="U")
        nc.vector.tensor_tensor(t_U[:, 0:NA], t_v0[:], t_v0[:], OP.mult)
        for m in range(1, 9):
            nc.vector.tensor_tensor(t_U[:, m * NA:(m + 1) * NA],
                                    t_U[:, (m - 1) * NA:m * NA],
                                    t_U[:, 0:NA], OP.mult)

        # wcz in turns (<=87), Veltkamp split
        t_wcz = p1.tile([NP, NZH], f32, tag="wcz")
        nc.vector.tensor_scalar(t_wcz[:], t_z[:], t_wct[:], None, OP.mult)
        t_wv = p1.tile([NP, NZH], f32, tag="wv")
        nc.vector.tensor_scalar(t_wv[:], t_wcz[:], VC, None, OP.mult)
        t_wz2 = p1.tile([NP, NZH], f32, tag="wz2")
        nc.vector.tensor_tensor(t_wz2[:], t_wv[:], t_wcz[:], OP.subtract)
        t_whi = p1.tile([NP, NZH], f32, tag="whi")
        nc.vector.tensor_tensor(t_whi[:], t_wv[:], t_wz2[:], OP.subtract)
        t_wlo = p1.tile([NP, NZH], f32, tag="wlo")
        nc.vector.tensor_tensor(t_wlo[:], t_wcz[:], t_whi[:], OP.subtract)

        # ---- flatten rows into matmul rhs tiles (SBUF->SBUF DMA) ----
        # sources rounded to f32r first; each rhs tile starts at partition 0
        t_Ur = p1.tile([NP, 9 * NA], f32r, tag="Ur")
        nc.vector.tensor_copy(t_Ur[:], t_U[:])
        t_khir = p1.tile([NP, NA], f32r, tag="khir")
        nc.vector.tensor_copy(t_khir[:], t_khi[:])
        t_klor = p1.tile([NP, NA], f32r, tag="klor")
        nc.vector.tensor_copy(t_klor[:], t_klo[:])
        t_ps1r = p1.tile([NP, NA], f32r, tag="ps1r")
        nc.vector.tensor_copy(t_ps1r[:], t_ps1[:])
        t_ps3r = p1.tile([NP, NA], f32r, tag="ps3r")
        nc.vector.tensor_copy(t_ps3r[:], t_ps3[:])
        t_a1r = p1.tile([NP, NA], f32r, tag="a1r")
        nc.vector.tensor_copy(t_a1r[:], t_a1[:])
        t_a2r = p1.tile([NP, NA], f32r, tag="a2r")
        nc.vector.tensor_copy(t_a2r[:], t_a2[:])
        t_1f = p1.tile([NP, NA], f32, tag="onesf")
        nc.vector.memset(t_1f[:], 1.0)
        t_1r = p1.tile([NP, NA], f32r, tag="onesr")
        nc.vector.tensor_copy(t_1r[:], t_1f[:])
        rowsP = p1.tile([10, F], f32r, tag="rowsP")   # ones | v^1..v^9
        nc.sync.dma_start(out=rowsP[0:1, :], in_=t_1r[:])
        for m in range(9):
            nc.sync.dma_start(out=rowsP[m + 1:m + 2, :],
                              in_=t_Ur[:, m * NA:(m + 1) * NA])
        rowsH = p1.tile([5, F], f32r, tag="rowsH")    # ones | khi klo khi klo
        nc.sync.dma_start(out=rowsH[0:1, :], in_=t_1r[:])
        nc.sync.dma_start(out=rowsH[1:2, :], in_=t_khir[:])
        nc.sync.dma_start(out=rowsH[2:3, :], in_=t_klor[:])
        nc.sync.dma_start(out=rowsH[3:4, :], in_=rowsH[1:2, :])
        nc.sync.dma_start(out=rowsH[4:5, :], in_=rowsH[2:3, :])
        rowsS = p1.tile([2, F], f32r, tag="rowsS")    # ps1 | ps3
        nc.sync.dma_start(out=rowsS[0:1, :], in_=t_ps1r[:])
        nc.sync.dma_start(out=rowsS[1:2, :], in_=t_ps3r[:])
        rowsA = p1.tile([2, F], f32r, tag="rowsA")    # a1 | a2
        nc.sync.dma_start(out=rowsA[0:1, :], in_=t_a1r[:])
        nc.sync.dma_start(out=rowsA[1:2, :], in_=t_a2r[:])

        # rowsC rows: 0 ones | 1 whi | 2 wlo | 3 whi | 4 wlo
        t_whir = p1.tile([NP, NZH], f32r, tag="whir")
        nc.vector.tensor_copy(t_whir[:], t_whi[:])
        t_wlor = p1.tile([NP, NZH], f32r, tag="wlor")
        nc.vector.tensor_copy(t_wlor[:], t_wlo[:])
        rowsC = p1.tile([5, FCH], f32r, tag="rowsC")
        nc.sync.dma_start(out=rowsC[0:1, :], in_=t_1r[:, 0:NZH])
        nc.sync.dma_start(out=rowsC[1:2, :], in_=t_whir[:])
        nc.sync.dma_start(out=rowsC[2:3, :], in_=t_wlor[:])
        nc.sync.dma_start(out=rowsC[3:4, :], in_=rowsC[1:2, :])
        nc.sync.dma_start(out=rowsC[4:5, :], in_=rowsC[2:3, :])

        # ---- field stage: per-chunk matmuls into PSUM + pointwise ----
        tJ0 = p1.tile([NJ, F], bf16, tag="J0")
        tMask = p1.tile([NJ, F], u8, tag="mask")
        tCT = p1.tile([NJ, NP * 26], bf16, tag="CT")
        ct3 = tCT[:].rearrange("p (n c) -> p n c", c=26)

        with tc.tile_pool(name="pf", bufs=2, space="PSUM") as pf:
            nchunks = (F + 511) // 512
            for c in range(nchunks):
                w = min(512, F - c * 512)
                sl = slice(c * 512, c * 512 + w)
                psT0 = pf.tile([NJ, 512], f32, tag="T0")
                nc.tensor.matmul(psT0[:, 0:w], t_lph1[:],
                                 rowsH[:, sl], start=True, stop=False)
                # mask from the pure x-part (before asymptotic corrections)
                nc.vector.tensor_scalar(tMask[:, sl], psT0[:, 0:w], TMASK,
                                        None, OP.is_le)
                nc.tensor.matmul(psT0[:, 0:w], t_lph2[:],
                                 rowsS[:, sl], start=False, stop=True)
                tRRm = p2.tile([NJ, 512], f32, tag="RRm")
                nc.scalar.activation(tRRm[:, 0:w], psT0[:, 0:w], AF.Copy,
                                     bias=MAGIC)
                tRR = p2.tile([NJ, 512], f32, tag="RR")
                nc.gpsimd.tensor_scalar(tRR[:, 0:w], tRRm[:, 0:w], -MAGIC,
                                        None, OP.add)
                tNU = p2.tile([NJ, 512], f32, tag="NU")
                nc.vector.tensor_tensor(tNU[:, 0:w], psT0[:, 0:w],
                                        tRR[:, 0:w], OP.subtract)
                tCOS = p2.tile([NJ, 512], f32, tag="COS")
                nc.scalar.activation(tCOS[:, 0:w], tNU[:, 0:w], AF.Sin,
                                     scale=2.0 * PI)
                psAMP = pf.tile([NJ, 512], f32, tag="AMP")
                nc.tensor.matmul(psAMP[:, 0:w], t_lamp[:],
                                 rowsA[:, sl], start=True, stop=True)
                nc.vector.tensor_tensor(tJ0[:, sl], psAMP[:, 0:w],
                                        tCOS[:, 0:w], OP.mult)
                psPOLY = pf.tile([NJ, 512], f32, tag="POLY")
                nc.tensor.matmul(psPOLY[:, 0:w], t_lpoly[:],
                                 rowsP[:, sl], start=True, stop=True)
                nc.vector.copy_predicated(tJ0[:, sl], tMask[:, sl],
                                          psPOLY[:, 0:w])

            # ---- C matrices: cos/sin(2pi * rho^2 * wcz) -> CT bf16 ----
            nb = [0, 38, 76, 114, NP]
            for ci in range(4):
                n0, n1 = nb[ci], nb[ci + 1]
                w = (n1 - n0) * NZH
                slc = slice(n0 * NZH, n0 * NZH + w)
                for lhs, zoff in ((t_lcos, 0), (t_lsin, NZH)):
                    psCC = pf.tile([NJ, 512], f32, tag="CC")
                    nc.tensor.matmul(psCC[:, 0:w], lhs[:],
                                     rowsC[:, slc], start=True, stop=True)
                    tCRm = p2.tile([NJ, 512], f32, tag="CRm")
                    nc.scalar.activation(tCRm[:, 0:w], psCC[:, 0:w], AF.Copy,
                                         bias=MAGIC)
                    tCRR = p2.tile([NJ, 512], f32, tag="CRR")
                    nc.gpsimd.tensor_scalar(tCRR[:, 0:w], tCRm[:, 0:w],
                                            -MAGIC, None, OP.add)
                    tCNU = p2.tile([NJ, 512], f32, tag="CNU")
                    nc.vector.tensor_tensor(tCNU[:, 0:w], psCC[:, 0:w],
                                            tCRR[:, 0:w], OP.subtract)
                    nc.scalar.activation(
                        ct3[:, n0:n1, zoff:zoff + NZH],
                        tCNU[:, 0:w].rearrange("p (n z) -> p n z", z=NZH),
                        AF.Sin, scale=2.0 * PI)

        # ---- per-pair contraction into PSUM, 4 waves of 32 pairs ----
        tUS = p1.tile([NA, FCH], f32, tag="plU")
        tVS = p1.tile([NA, FCH], f32, tag="plV")
        tPL = p1.tile([NA, FCH], f32r, tag="plP")
        us_z = tUS[:].rearrange("q (zz pp) -> q pp zz", pp=NP)
        vs_z = tVS[:].rearrange("q (zz pp) -> q pp zz", pp=NP)
        WP = 32
        with tc.tile_pool(name="ppr", bufs=2, space="PSUM") as ppr, \
                tc.tile_pool(name="pso", bufs=1, space="PSUM") as pso, \
                tc.tile_pool(name="pgo", bufs=2, space="PSUM") as pgo:
            for wv in range(NP // WP):
                tPRw = ppr.tile([NA, WP * 32], f32, tag="PR")
                for j in range(WP):
                    p = wv * WP + j
                    nc.tensor.matmul(tPRw[:, j * 32:j * 32 + 26],
                                     tJ0[:, p * NA:(p + 1) * NA],
                                     tCT[:, p * 26:(p + 1) * 26],
                                     start=True, stop=True)
                pr4 = tPRw[:].rearrange("q (n s) -> q n s", s=32)
                slw = slice(wv * WP, (wv + 1) * WP)
                nc.scalar.activation(us_z[:, slw, :], pr4[:, :, 0:NZH],
                                     AF.Square)
                nc.scalar.activation(vs_z[:, slw, :], pr4[:, :, NZH:26],
                                     AF.Square)
            nc.vector.tensor_tensor(tPL[:], tUS[:], tVS[:], OP.add)

            # ---- normalization: nrm[p] = sum_zz sum_a gw13[a,zz]*PL[a,zz*NP+p]
            psN = pso.tile([NP, 2], f32, tag="N")
            for zz in range(NZH):
                nc.tensor.matmul(psN[:], tPL[:, zz * NP:(zz + 1) * NP],
                                 t_gw[:, 2 * zz:2 * zz + 2],
                                 start=(zz == 0), stop=(zz == NZH - 1))
            tRC = p1.tile([NP, 1], f32, tag="RC")
            nc.vector.reciprocal(tRC[:], psN[:, 0:1])

            # ---- G expansion + normalize-on-copy + mirrored output ----
            for zz in range(NZH):
                lhs = tPL[:, zz * NP:(zz + 1) * NP]
                for h in range(2):
                    wcols = NYX - h * 320 if h == 1 else 320  # 320, 305
                    tOC = pgo.tile([NP, 320], f32, tag="OC")
                    nc.tensor.matmul(tOC[:], lhs,
                                     t_g[:, h * 320:(h + 1) * 320],
                                     start=True, stop=True)
                    tOS = p2.tile([NP, 320], f32, tag="OS")
                    if h == 0:
                        nc.scalar.activation(tOS[:], tOC[:], AF.Copy,
                                             scale=tRC[:, 0:1])
                    else:
                        nc.vector.tensor_scalar(tOS[:], tOC[:], tRC[:, 0:1],
                                                None, OP.mult)
                    nc.sync.dma_start(
                        out=d_out[:, 12 + zz, h * 320:h * 320 + wcols],
                        in_=tOS[:, 0:wcols])
                    if zz > 0:
                        nc.sync.dma_start(
                            out=d_out[:, 12 - zz, h * 320:h * 320 + wcols],
                            in_=tOS[:, 0:wcols])

    nc.finalize()
    _CACHE["nc"] = nc
    return nc


def _in_maps(params, consts):
    per = B // NCORES
    maps = []
    for i in range(NCORES):
        m = {"params": params[i * per:(i + 1) * per].reshape(NP, 2).copy()}
        m.update(consts)
        maps.append(m)
    return maps


def kernel(params):
    _ensure_paths()
    from concourse.bass_utils import run_bass_kernel_spmd

    params = np.asarray(params, dtype=np.float32)
    assert params.shape == (B, CH, 2)
    consts = _host_consts()
    nc = _build_nc()
    res = run_bass_kernel_spmd(nc, _in_maps(params, consts),
                               list(range(NCORES)))
    per = B // NCORES
    out = np.empty((B, CH, NZ, 25, 25), np.float32)
    for i in range(NCORES):
        out[i * per:(i + 1) * per] = res.results[i]["out"].reshape(
            per, CH, NZ, 25, 25)
    return out


def kernel_traced(params, tmpdir=None):
    """Run once with NTFF tracing; returns HW exec_time_ns (slowest core)."""
    _ensure_paths()
    from concourse.bass_utils import run_bass_kernel_spmd

    params = np.asarray(params, dtype=np.float32)
    consts = _host_consts()
    nc = _build_nc()
    res = run_bass_kernel_spmd(nc, _in_maps(params, consts),
                               list(range(NCORES)), trace=True, tmpdir=tmpdir)
    return res.exec_time_ns


# revision 15
# speedup vs baseline: 1.6370x; 1.6370x over previous
"""Born-Wolf PSF kernel for Trainium2, 8 NeuronCores, data-parallel over batch.

Self-contained: hardcodes all geometry from the problem spec.
  input : params (16, 64, 2) float32
  output: psf    (16, 64, 25, 25, 25) float32

Per (b,c) pair: psf = |trapz_rho J0(k n r rho) exp(-i 0.5 k rho^2 z n^2) rho|^2,
bilinearly interpolated from 35 anchor radii onto a 25x25 grid, reflect-padded
in z, and normalized.

v2 strategy: all per-(rho, pair*anchor) field quantities are sums of outer
products (rank-k separable), so they are computed by TensorE matmuls directly
into PSUM instead of DRAM-row broadcast DMAs (which serialized ~375us on one
DMA engine in v1). f32r matmuls truncate inputs to ~12 mantissa bits, so the
large phase factors (up to ~145 turns) are split hi/lo: hi holds 10 explicit
bits (exact in f32r), lo carries the remainder; the rank-5 expansion
(ones, hi*hi, hi*lo, lo*hi, lo*lo) restores full fp32 accuracy at 1 cycle/col.
Trapezoid weights and 1/2pi factors are folded into the matmul lhs constants;
normalization is folded into the per-partition scale of the PSUM->SBUF copy
after the G-expansion matmul.
"""
import os
import numpy as np

# ---------------- problem geometry (hardcoded) ----------------
B, CH = 16, 64
NCORES = 8
NP = (B // NCORES) * CH          # 128 pairs per core
NA, NJ, NZH, NZ = 35, 101, 13, 25
F = NP * NA                      # 4480
FCH = NP * NZH                   # 1664
NYX = 625
GW = 640                         # zero-padded G columns (2 x 320 matmuls)
PI = float(np.pi)
C0 = -0.1562499995e-1
C1 = -0.1098628627e-2
C2 = 0.1430488765e-3
S_AMP = float(np.sqrt(0.636619772))
SC1 = 20.0                       # power-row scale split (clamp = SC1^2 = 400)
QS = [1.0, -0.25, 0.015624999996, -0.00043402777473, 6.7816828549e-06,
      -6.781657507e-08, 4.7091319698e-10, -2.3995591574e-12,
      9.2118377553e-15, -2.3695100804e-17]
MAGIC = 12582912.0               # 1.5 * 2**23: (t+M)-M == round-to-nearest(t)
TMASK = 4.0 / (2.0 * PI) + 0.125  # mask threshold in turn units
VC = 8193.0                      # Veltkamp split const (2^13+1): hi keeps 10 bits

_CACHE = {}


def _split10(x):
    """Split f32 values into (hi, lo); hi has <=10 explicit mantissa bits."""
    xf = np.ascontiguousarray(np.asarray(x, np.float32))
    hi = (xf.view(np.uint32) & np.uint32(0xFFFFE000)).view(np.float32).copy()
    lo = (xf - hi).astype(np.float32)
    return hi, lo


def _host_consts():
    if "consts" in _CACHE:
        return _CACHE["consts"]
    f32 = np.float32
    R = (np.linspace(0, 34, NA) / 2.0).astype(np.float64)          # anchor radii
    RHO = np.linspace(0.0, 1.0, NJ).astype(np.float64)
    yp = xp = 12.0
    Y, X = np.meshgrid(np.arange(25.0), np.arange(25.0), indexing="ij")
    rPix = np.sqrt((X - xp) ** 2 + (Y - yp) ** 2)
    IDX1 = np.floor(rPix * 2).astype(np.int32)
    IDX2 = IDX1 + 1
    DISR1 = ((rPix - R[IDX1]) * 2).astype(np.float32).astype(np.float64)
    DISR2 = 1.0 - DISR1

    G = np.zeros((NA, GW), np.float64)
    for yy in range(25):
        for xx in range(25):
            yx = yy * 25 + xx
            G[IDX2[yy, xx], yx] += DISR1[yy, xx]
            G[IDX1[yy, xx], yx] += DISR2[yy, xx]
    gcol = G[:, :NYX].sum(1)
    w13 = np.concatenate([[1.0], np.full(NZH - 1, 2.0)])
    gw13 = np.zeros((NA, 2 * NZH))
    gw13[:, 0::2] = gcol[:, None] * w13[None, :]

    wt = np.full(NJ, 0.01, np.float64)
    wt[0] *= 0.5
    wt[-1] *= 0.5
    rw = RHO * wt                                    # trapezoid weight * rho

    with np.errstate(divide="ignore"):
        rinv = 1.0 / RHO
    rinv[0] = 0.0

    # phase lhs: t0 = 0.125 + RHO*knr_t + ps1*rinv/(2pi)^2 + ps3*rinv^3/(2pi)^4
    rho_hi, rho_lo = _split10(RHO)
    L_ph1 = np.stack([np.full(NJ, 0.125), rho_hi, rho_hi, rho_lo, rho_lo])
    L_ph2 = np.stack([rinv / (2 * np.pi) ** 2, rinv ** 3 / (2 * np.pi) ** 4])

    # poly lhs (J0 small branch, trapezoid weight folded in); row 0 <-> ones
    qrho = np.stack([QS[m] * (SC1 * SC1 * RHO ** 2) ** m for m in range(10)])
    L_poly = qrho * rw[None, :]

    # amplitude lhs (J0 large branch, trapezoid weight folded in)
    L_amp = np.stack([np.sqrt(rinv) * rw, -rinv ** 2.5 * rw])

    # C-matrix lhs: turns = RHO^2 * wcz_t  (+0.25 for cos; negated for sin)
    r2hi, r2lo = _split10(RHO ** 2)
    L_cos = np.stack([np.full(NJ, 0.25), r2hi, r2hi, r2lo, r2lo])
    L_sin = np.stack([np.zeros(NJ), -r2hi, -r2hi, -r2lo, -r2lo])

    Rinv = 1.0 / np.maximum(R, 1e-9)
    Rinv[0] = 0.0
    rtab = np.tile(R[None, :], (NP, 1))
    ri8c0 = np.tile((8.0 * C0 * Rinv)[None, :], (NP, 1))
    ri3c2 = np.tile((512.0 * C2 * Rinv ** 3)[None, :], (NP, 1))
    ztab = np.tile(np.arange(NZH, dtype=np.float64)[None, :], (NP, 1))

    consts = {
        "Lph1": L_ph1.astype(f32),
        "Lph2": L_ph2.astype(f32),
        "Lpoly": L_poly.astype(f32),
        "Lamp": L_amp.astype(f32),
        "Lcos": L_cos.astype(f32),
        "Lsin": L_sin.astype(f32),
        "rtab": rtab.astype(f32),
        "ri8c0": ri8c0.astype(f32),
        "ri3c2": ri3c2.astype(f32),
        "ztab": ztab.astype(f32),
        "gpad": G.astype(f32),
        "gw13": gw13.astype(f32),
    }
    for k, v in consts.items():
        assert np.isfinite(v).all(), k
    _CACHE["consts"] = consts
    return consts


def _ensure_paths():
    import sys
    for p in ("/opt/trn_rl_repo", "/root/.axon_site/_ro/trn_rl_repo"):
        if os.path.isdir(p) and p not in sys.path:
            sys.path.append(p)


def _build_nc():
    if "nc" in _CACHE:
        return _CACHE["nc"]
    _ensure_paths()
    from contextlib import ExitStack
    import concourse.bass as bass
    import concourse.bacc as bacc
    import concourse.tile as tile
    from concourse import mybir

    f32 = mybir.dt.float32
    f32r = mybir.dt.float32r
    bf16 = mybir.dt.bfloat16
    u8 = mybir.dt.uint8
    AF = mybir.ActivationFunctionType
    OP = mybir.AluOpType

    nc = bacc.Bacc()
    BIAS_A1 = float(np.log(S_AMP) - 0.5 * np.log(2 * np.pi))
    BIAS_A2 = float(np.log(64.0 * abs(C1) * S_AMP) - 2.5 * np.log(2 * np.pi))
    for val in (BIAS_A1, BIAS_A2):
        t = nc.alloc_sbuf_tensor(f"const-f32-{val}", [128, 1], f32)
        nc.gpsimd.memset(t.ap(), val)
        nc.const_aps.aps[(f32, val)] = t.ap()
    nc.all_engine_barrier()

    d_par = nc.declare_dram_parameter("params", [NP, 2], f32, isOutput=False)
    d_lph1 = nc.declare_dram_parameter("Lph1", [5, NJ], f32, isOutput=False)
    d_lph2 = nc.declare_dram_parameter("Lph2", [2, NJ], f32, isOutput=False)
    d_lpoly = nc.declare_dram_parameter("Lpoly", [10, NJ], f32, isOutput=False)
    d_lamp = nc.declare_dram_parameter("Lamp", [2, NJ], f32, isOutput=False)
    d_lcos = nc.declare_dram_parameter("Lcos", [5, NJ], f32, isOutput=False)
    d_lsin = nc.declare_dram_parameter("Lsin", [5, NJ], f32, isOutput=False)
    d_rtab = nc.declare_dram_parameter("rtab", [NP, NA], f32, isOutput=False)
    d_ri1 = nc.declare_dram_parameter("ri8c0", [NP, NA], f32, isOutput=False)
    d_ri3 = nc.declare_dram_parameter("ri3c2", [NP, NA], f32, isOutput=False)
    d_z = nc.declare_dram_parameter("ztab", [NP, NZH], f32, isOutput=False)
    d_g = nc.declare_dram_parameter("gpad", [NA, GW], f32, isOutput=False)
    d_gw = nc.declare_dram_parameter("gw13", [NA, 2 * NZH], f32,
                                     isOutput=False)
    d_out = nc.declare_dram_parameter("out", [NP, NZ, NYX], f32, isOutput=True)

    with tile.TileContext(nc) as tc, ExitStack() as ctx:
        p1 = ctx.enter_context(tc.tile_pool(name="p1", bufs=1))
        p2 = ctx.enter_context(tc.tile_pool(name="p2", bufs=3))

        # ---- const loads ----
        t_par = p1.tile([NP, 2], f32, tag="par")
        t_lph1f = p1.tile([5, NJ], f32, tag="lph1f")
        t_lph2f = p1.tile([2, NJ], f32, tag="lph2f")
        t_lpolyf = p1.tile([10, NJ], f32, tag="lpolyf")
        t_lampf = p1.tile([2, NJ], f32, tag="lampf")
        t_lcosf = p1.tile([5, NJ], f32, tag="lcosf")
        t_lsinf = p1.tile([5, NJ], f32, tag="lsinf")
        t_rtab = p1.tile([NP, NA], f32, tag="rtab")
        t_ri1 = p1.tile([NP, NA], f32, tag="ri1")
        t_ri3 = p1.tile([NP, NA], f32, tag="ri3")
        t_z = p1.tile([NP, NZH], f32, tag="ztab")
        t_gf = p1.tile([NA, GW], f32, tag="gpadf")
        t_gwf = p1.tile([NA, 2 * NZH], f32, tag="gw13f")
        for t, d in ((t_par, d_par), (t_lph1f, d_lph1), (t_lph2f, d_lph2),
                     (t_lpolyf, d_lpoly), (t_lampf, d_lamp), (t_lcosf, d_lcos),
                     (t_lsinf, d_lsin), (t_rtab, d_rtab), (t_ri1, d_ri1),
                     (t_ri3, d_ri3), (t_z, d_z), (t_gf, d_g), (t_gwf, d_gw)):
            nc.sync.dma_start(out=t[:], in_=d[:])
        t_lph1 = p1.tile([5, NJ], f32r, tag="lph1")
        t_lph2 = p1.tile([2, NJ], f32r, tag="lph2")
        t_lpoly = p1.tile([10, NJ], f32r, tag="lpoly")
        t_lamp = p1.tile([2, NJ], f32r, tag="lamp")
        t_lcos = p1.tile([5, NJ], f32r, tag="lcos")
        t_lsin = p1.tile([5, NJ], f32r, tag="lsin")
        t_g = p1.tile([NA, GW], f32r, tag="gpad")
        t_gw = p1.tile([NA, 2 * NZH], f32r, tag="gw13")
        for dst, srcf in ((t_lph1, t_lph1f), (t_lph2, t_lph2f),
                          (t_lpoly, t_lpolyf), (t_lamp, t_lampf),
                          (t_lcos, t_lcosf), (t_lsin, t_lsinf),
                          (t_g, t_gf), (t_gw, t_gwf)):
            nc.vector.tensor_copy(dst[:], srcf[:])

        # ---- pair-scalar stage ([NP,1] / [NP,NA]) ----
        t_abs = p1.tile([NP, 2], f32, tag="pabs")
        nc.vector.scalar_tensor_tensor(t_abs[:], t_par[:], -1.0, t_par[:],
                                       OP.mult, OP.max)
        lam = t_abs[:, 0:1]
        enn = t_abs[:, 1:2]
        t_rl = p1.tile([NP, 1], f32, tag="rl")
        nc.vector.reciprocal(t_rl[:], lam)
        t_knt = p1.tile([NP, 1], f32, tag="knt")       # n/lam (turns per R*rho)
        nc.vector.tensor_tensor(t_knt[:], enn, t_rl[:], OP.mult)
        t_rkn = p1.tile([NP, 1], f32, tag="rkn")       # lam/n
        nc.vector.reciprocal(t_rkn[:], t_knt[:])
        t_rkn3 = p1.tile([NP, 1], f32, tag="rkn3")
        nc.vector.tensor_tensor(t_rkn3[:], t_rkn[:], t_rkn[:], OP.mult)
        nc.vector.tensor_tensor(t_rkn3[:], t_rkn3[:], t_rkn[:], OP.mult)
        t_wct = p1.tile([NP, 1], f32, tag="wct")       # 0.5*n^2/lam
        nc.vector.scalar_tensor_tensor(t_wct[:], enn, 0.5, t_knt[:],
                                       OP.mult, OP.mult)

        t_knr = p1.tile([NP, NA], f32, tag="knr")      # knr in turns, <=145
        nc.vector.tensor_scalar(t_knr[:], t_rtab[:], t_knt[:], None, OP.mult)
        # Veltkamp split: hi keeps ~10 bits (exact under f32r truncation)
        t_kv = p1.tile([NP, NA], f32, tag="kv")
        nc.vector.tensor_scalar(t_kv[:], t_knr[:], VC, None, OP.mult)
        t_kz = p1.tile([NP, NA], f32, tag="kz")
        nc.vector.tensor_tensor(t_kz[:], t_kv[:], t_knr[:], OP.subtract)
        t_khi = p1.tile([NP, NA], f32, tag="khi")
        nc.vector.tensor_tensor(t_khi[:], t_kv[:], t_kz[:], OP.subtract)
        t_klo = p1.tile([NP, NA], f32, tag="klo")
        nc.vector.tensor_tensor(t_klo[:], t_knr[:], t_khi[:], OP.subtract)

        t_ps1 = p1.tile([NP, NA], f32, tag="ps1")
        nc.vector.tensor_scalar(t_ps1[:], t_ri1[:], t_rkn[:], None, OP.mult)
        t_ps3 = p1.tile([NP, NA], f32, tag="ps3")
        nc.vector.tensor_scalar(t_ps3[:], t_ri3[:], t_rkn3[:], None, OP.mult)

        t_knm = p1.tile([NP, NA], f32, tag="knm")
        nc.vector.tensor_scalar_max(t_knm[:], t_knr[:], 1e-4)
        t_lk = p1.tile([NP, NA], f32, tag="lk")
        nc.scalar.activation(t_lk[:], t_knm[:], AF.Ln)
        t_a1 = p1.tile([NP, NA], f32, tag="a1")
        nc.scalar.activation(t_a1[:], t_lk[:], AF.Exp, bias=BIAS_A1, scale=-0.5)
        t_a2 = p1.tile([NP, NA], f32, tag="a2")
        nc.scalar.activation(t_a2[:], t_lk[:], AF.Exp, bias=BIAS_A2, scale=-2.5)

        # power rows: v = min(knr_rad/SC1, SC1)^2 ; U[:, m*NA:(m+1)*NA] = v^(m+1)
        t_v0 = p1.tile([NP, NA], f32, tag="v0")
        nc.vector.tensor_scalar(t_v0[:], t_knr[:], 2.0 * PI / SC1, SC1,
                                OP.mult, OP.min)
        t_U = p1.tile([NP, 9 * NA], f32, tag="U")
        nc.vector.tensor_tensor(t_U[:, 0:NA], t_v0[:], t_v0[:], OP.mult)
        for m in range(1, 9):
            nc.vector.tensor_tensor(t_U[:, m * NA:(m + 1) * NA],
                                    t_U[:, (m - 1) * NA:m * NA],
                                    t_U[:, 0:NA], OP.mult)

        # wcz in turns (<=87), Veltkamp split
        t_wcz = p1.tile([NP, NZH], f32, tag="wcz")
        nc.vector.tensor_scalar(t_wcz[:], t_z[:], t_wct[:], None, OP.mult)
        t_wv = p1.tile([NP, NZH], f32, tag="wv")
        nc.vector.tensor_scalar(t_wv[:], t_wcz[:], VC, None, OP.mult)
        t_wz2 = p1.tile([NP, NZH], f32, tag="wz2")
        nc.vector.tensor_tensor(t_wz2[:], t_wv[:], t_wcz[:], OP.subtract)
        t_whi = p1.tile([NP, NZH], f32, tag="whi")
        nc.vector.tensor_tensor(t_whi[:], t_wv[:], t_wz2[:], OP.subtract)
        t_wlo = p1.tile([NP, NZH], f32, tag="wlo")
        nc.vector.tensor_tensor(t_wlo[:], t_wcz[:], t_whi[:], OP.subtract)

        # ---- flatten rows into matmul rhs tiles (SBUF->SBUF DMA) ----
        # sources rounded to f32r first; each rhs tile starts at partition 0
        t_Ur = p1.tile([NP, 9 * NA], f32r, tag="Ur")
        nc.vector.tensor_copy(t_Ur[:], t_U[:])
        t_khir = p1.tile([NP, NA], f32r, tag="khir")
        nc.vector.tensor_copy(t_khir[:], t_khi[:])
        t_klor = p1.tile([NP, NA], f32r, tag="klor")
        nc.vector.tensor_copy(t_klor[:], t_klo[:])
        t_ps1r = p1.tile([NP, NA], f32r, tag="ps1r")
        nc.vector.tensor_copy(t_ps1r[:], t_ps1[:])
        t_ps3r = p1.tile([NP, NA], f32r, tag="ps3r")
        nc.vector.tensor_copy(t_ps3r[:], t_ps3[:])
        t_a1r = p1.tile([NP, NA], f32r, tag="a1r")
        nc.vector.tensor_copy(t_a1r[:], t_a1[:])
        t_a2r = p1.tile([NP, NA], f32r, tag="a2r")
        nc.vector.tensor_copy(t_a2r[:], t_a2[:])
        t_1f = p1.tile([NP, NA], f32, tag="onesf")
        nc.vector.memset(t_1f[:], 1.0)
        t_1r = p1.tile([NP, NA], f32r, tag="onesr")
        nc.vector.tensor_copy(t_1r[:], t_1f[:])
        rowsP = p1.tile([10, F], f32r, tag="rowsP")   # ones | v^1..v^9
        nc.sync.dma_start(out=rowsP[0:1, :], in_=t_1r[:])
        for m in range(9):
            nc.sync.dma_start(out=rowsP[m + 1:m + 2, :],
                              in_=t_Ur[:, m * NA:(m + 1) * NA])
        rowsH = p1.tile([5, F], f32r, tag="rowsH")    # ones | khi klo khi klo
        nc.sync.dma_start(out=rowsH[0:1, :], in_=t_1r[:])
        nc.sync.dma_start(out=rowsH[1:2, :], in_=t_khir[:])
        nc.sync.dma_start(out=rowsH[2:3, :], in_=t_klor[:])
        nc.sync.dma_start(out=rowsH[3:4, :], in_=rowsH[1:2, :])
        nc.sync.dma_start(out=rowsH[4:5, :], in_=rowsH[2:3, :])
        rowsS = p1.tile([2, F], f32r, tag="rowsS")    # ps1 | ps3
        nc.sync.dma_start(out=rowsS[0:1, :], in_=t_ps1r[:])
        nc.sync.dma_start(out=rowsS[1:2, :], in_=t_ps3r[:])
        rowsA = p1.tile([2, F], f32r, tag="rowsA")    # a1 | a2
        nc.sync.dma_start(out=rowsA[0:1, :], in_=t_a1r[:])
        nc.sync.dma_start(out=rowsA[1:2, :], in_=t_a2r[:])

        # rowsC rows: 0 ones | 1 whi | 2 wlo | 3 whi | 4 wlo
        t_whir = p1.tile([NP, NZH], f32r, tag="whir")
        nc.vector.tensor_copy(t_whir[:], t_whi[:])
        t_wlor = p1.tile([NP, NZH], f32r, tag="wlor")
        nc.vector.tensor_copy(t_wlor[:], t_wlo[:])
        rowsC = p1.tile([5, FCH], f32r, tag="rowsC")
        nc.sync.dma_start(out=rowsC[0:1, :], in_=t_1r[:, 0:NZH])
        nc.sync.dma_start(out=rowsC[1:2, :], in_=t_whir[:])
        nc.sync.dma_start(out=rowsC[2:3, :], in_=t_wlor[:])
        nc.sync.dma_start(out=rowsC[3:4, :], in_=rowsC[1:2, :])
        nc.sync.dma_start(out=rowsC[4:5, :], in_=rowsC[2:3, :])

        # ---- field stage: per-chunk matmuls into PSUM + pointwise ----
        tJ0 = p1.tile([NJ, F], bf16, tag="J0")
        tMask = p1.tile([NJ, F], u8, tag="mask")
        tCT = p1.tile([NJ, NP * 26], bf16, tag="CT")
        ct3 = tCT[:].rearrange("p (n c) -> p n c", c=26)

        CW = 1024
        with tc.tile_pool(name="pf", bufs=2, space="PSUM") as pf:
            nchunks = (F + CW - 1) // CW
            for c in range(nchunks):
                w = min(CW, F - c * CW)
                sl = slice(c * CW, c * CW + w)
                hws = [(0, min(512, w))] + ([(512, w - 512)] if w > 512
                                             else [])
                psT0 = pf.tile([NJ, CW], f32, tag="T0")
                for o, hw in hws:
                    nc.tensor.matmul(psT0[:, o:o + hw], t_lph1[:],
                                     rowsH[:, c * CW + o:c * CW + o + hw],
                                     start=True, stop=False)
                # mask from the pure x-part (before asymptotic corrections)
                nc.vector.tensor_scalar(tMask[:, sl], psT0[:, 0:w], TMASK,
                                        None, OP.is_le)
                for o, hw in hws:
                    nc.tensor.matmul(psT0[:, o:o + hw], t_lph2[:],
                                     rowsS[:, c * CW + o:c * CW + o + hw],
                                     start=False, stop=True)
                tRRm = p2.tile([NJ, CW], f32, tag="RRm")
                nc.scalar.activation(tRRm[:, 0:w], psT0[:, 0:w], AF.Copy,
                                     bias=MAGIC)
                tRR = p2.tile([NJ, CW], f32, tag="RR")
                nc.scalar.activation(tRR[:, 0:w], tRRm[:, 0:w], AF.Copy,
                                     bias=-MAGIC)
                tNU = p2.tile([NJ, CW], f32, tag="NU")
                nc.vector.tensor_tensor(tNU[:, 0:w], psT0[:, 0:w],
                                        tRR[:, 0:w], OP.subtract)
                tCOS = p2.tile([NJ, CW], f32, tag="COS")
                nc.scalar.activation(tCOS[:, 0:w], tNU[:, 0:w], AF.Sin,
                                     scale=2.0 * PI)
                psX = pf.tile([NJ, CW], f32, tag="X")
                for o, hw in hws:
                    nc.tensor.matmul(psX[:, o:o + hw], t_lamp[:],
                                     rowsA[:, c * CW + o:c * CW + o + hw],
                                     start=True, stop=True)
                nc.vector.tensor_tensor(tJ0[:, sl], psX[:, 0:w],
                                        tCOS[:, 0:w], OP.mult)
                for o, hw in hws:
                    nc.tensor.matmul(psX[:, o:o + hw], t_lpoly[:],
                                     rowsP[:, c * CW + o:c * CW + o + hw],
                                     start=True, stop=True)
                nc.vector.copy_predicated(tJ0[:, sl], tMask[:, sl],
                                          psX[:, 0:w])

            # ---- C matrices: cos/sin(2pi * rho^2 * wcz) -> CT bf16 ----
            nb = [0, 78, NP]
            for ci in range(2):
                n0, n1 = nb[ci], nb[ci + 1]
                w = (n1 - n0) * NZH
                slc = slice(n0 * NZH, n0 * NZH + w)
                chws = [(0, min(512, w))] + ([(512, w - 512)] if w > 512
                                              else [])
                for lhs, zoff, ptag in ((t_lcos, 0, "T0"), (t_lsin, NZH, "X")):
                    psCC = pf.tile([NJ, CW], f32, tag=ptag)
                    for o, hw in chws:
                        nc.tensor.matmul(
                            psCC[:, o:o + hw], lhs[:],
                            rowsC[:, n0 * NZH + o:n0 * NZH + o + hw],
                            start=True, stop=True)
                    tCRm = p2.tile([NJ, CW], f32, tag="RRm")
                    nc.scalar.activation(tCRm[:, 0:w], psCC[:, 0:w], AF.Copy,
                                         bias=MAGIC)
                    tCRR = p2.tile([NJ, CW], f32, tag="RR")
                    nc.scalar.activation(tCRR[:, 0:w], tCRm[:, 0:w], AF.Copy,
                                         bias=-MAGIC)
                    tCNU = p2.tile([NJ, CW], f32, tag="NU")
                    nc.vector.tensor_tensor(tCNU[:, 0:w], psCC[:, 0:w],
                                            tCRR[:, 0:w], OP.subtract)
                    nc.scalar.activation(
                        ct3[:, n0:n1, zoff:zoff + NZH],
                        tCNU[:, 0:w].rearrange("p (n z) -> p n z", z=NZH),
                        AF.Sin, scale=2.0 * PI)

        # ---- per-pair contraction into PSUM, 4 waves of 32 pairs ----
        tUS = p1.tile([NA, FCH], f32, tag="plU")
        tVS = p1.tile([NA, FCH], f32, tag="plV")
        tPL = p1.tile([NA, FCH], f32r, tag="plP")
        us_z = tUS[:].rearrange("q (zz pp) -> q pp zz", pp=NP)
        vs_z = tVS[:].rearrange("q (zz pp) -> q pp zz", pp=NP)
        WP = 32
        with tc.tile_pool(name="ppr", bufs=2, space="PSUM") as ppr, \
                tc.tile_pool(name="pso", bufs=1, space="PSUM") as pso, \
                tc.tile_pool(name="pgo", bufs=2, space="PSUM") as pgo:
            for wv in range(NP // WP):
                tPRw = ppr.tile([NA, WP * 32], f32, tag="PR")
                for j in range(WP):
                    p = wv * WP + j
                    nc.tensor.matmul(tPRw[:, j * 32:j * 32 + 26],
                                     tJ0[:, p * NA:(p + 1) * NA],
                                     tCT[:, p * 26:(p + 1) * 26],
                                     start=True, stop=True)
                pr4 = tPRw[:].rearrange("q (n s) -> q n s", s=32)
                slw = slice(wv * WP, (wv + 1) * WP)
                nc.scalar.activation(us_z[:, slw, :], pr4[:, :, 0:NZH],
                                     AF.Square)
                nc.scalar.activation(vs_z[:, slw, :], pr4[:, :, NZH:26],
                                     AF.Square)
            nc.vector.tensor_tensor(tPL[:], tUS[:], tVS[:], OP.add)

            # ---- normalization: nrm[p] = sum_zz sum_a gw13[a,zz]*PL[a,zz*NP+p]
            psN = pso.tile([NP, 2], f32, tag="N")
            for zz in range(NZH):
                nc.tensor.matmul(psN[:], tPL[:, zz * NP:(zz + 1) * NP],
                                 t_gw[:, 2 * zz:2 * zz + 2],
                                 start=(zz == 0), stop=(zz == NZH - 1))
            tRC = p1.tile([NP, 1], f32, tag="RC")
            nc.vector.reciprocal(tRC[:], psN[:, 0:1])

            # ---- G expansion + normalize-on-copy + mirrored output ----
            for zz in range(NZH):
                lhs = tPL[:, zz * NP:(zz + 1) * NP]
                for h in range(2):
                    wcols = NYX - h * 320 if h == 1 else 320  # 320, 305
                    tOC = pgo.tile([NP, 320], f32, tag="OC")
                    nc.tensor.matmul(tOC[:], lhs,
                                     t_g[:, h * 320:(h + 1) * 320],
                                     start=True, stop=True)
                    tOS = p2.tile([NP, 320], f32, tag="OS")
                    if h == 0:
                        nc.scalar.activation(tOS[:], tOC[:], AF.Copy,
                                             scale=tRC[:, 0:1])
                    else:
                        nc.vector.tensor_scalar(tOS[:], tOC[:], tRC[:, 0:1],
                                                None, OP.mult)
                    nc.sync.dma_start(
                        out=d_out[:, 12 + zz, h * 320:h * 320 + wcols],
                        in_=tOS[:, 0:wcols])
                    if zz > 0:
                        nc.sync.dma_start(
                            out=d_out[:, 12 - zz, h * 320:h * 320 + wcols],
                            in_=tOS[:, 0:wcols])

    nc.finalize()
    _CACHE["nc"] = nc
    return nc


def _in_maps(params, consts):
    per = B // NCORES
    maps = []
    for i in range(NCORES):
        m = {"params": params[i * per:(i + 1) * per].reshape(NP, 2).copy()}
        m.update(consts)
        maps.append(m)
    return maps


def kernel(params):
    _ensure_paths()
    from concourse.bass_utils import run_bass_kernel_spmd

    params = np.asarray(params, dtype=np.float32)
    assert params.shape == (B, CH, 2)
    consts = _host_consts()
    nc = _build_nc()
    res = run_bass_kernel_spmd(nc, _in_maps(params, consts),
                               list(range(NCORES)))
    per = B // NCORES
    out = np.empty((B, CH, NZ, 25, 25), np.float32)
    for i in range(NCORES):
        out[i * per:(i + 1) * per] = res.results[i]["out"].reshape(
            per, CH, NZ, 25, 25)
    return out


def kernel_traced(params, tmpdir=None):
    """Run once with NTFF tracing; returns HW exec_time_ns (slowest core)."""
    _ensure_paths()
    from concourse.bass_utils import run_bass_kernel_spmd

    params = np.asarray(params, dtype=np.float32)
    consts = _host_consts()
    nc = _build_nc()
    res = run_bass_kernel_spmd(nc, _in_maps(params, consts),
                               list(range(NCORES)), trace=True, tmpdir=tmpdir)
    return res.exec_time_ns
